# revision 1
# baseline (speedup 1.0000x reference)
"""DGCNN classification kernel for 8x Trainium2 NeuronCores (v2).

Data-parallel: one point cloud (N=1024 points) per core, 8 clouds total.

EdgeConv restructuring (numerically equivalent to the reference):
  max_k LeakyReLU(BN(W @ [h_j; h_i])) = LeakyReLU(max_j (A1 h_j) + (A2 h_i + c))
with A = diag(bn_scale) W and c the folded BN shift (LeakyReLU commutes
with max).

v2 changes vs the 816us baseline:
  * top-20 selection via index-embedded scores: w = round_1024(alpha*(-0.5 d^2))
    + j, built with one ACT pass (bias 3*2^32 forces fp32 rounding to 1024 in
    the [2^33,2^34) binade), an ACT Relu clamp at -(2^23-1024), and one DVE
    scalar_tensor_tensor iota add. 3x max8 + 2x match_replace then yield the
    top-24 *values with their indices in the low bits* -- the three
    FIND_INDEX8 full scans of the baseline are gone. Indices are recovered
    with a 5-op chain on [128, 192] tiles.
  * combine phase on PE+ACT: zT = transpose(kmax) + A2 h (PSUM accumulate),
    h_next = Lrelu(zT + c) on ACT with per-partition bias -- no DVE work.
  * k-max reduce split between DVE and GpSimd.
  * index wrap build in fp16 (PE single-pass instead of fp32 LOW_HIGH).

Scores/U/V stay fp32: the knn selection is chaotically sensitive; tf32/bf16
scores push the end-to-end error past the 2e-2 gate (host-validated). The
alpha quantization gives ~9e-3.
"""

import os
import sys

import numpy as np

sys.path.insert(0, "/opt/trn_rl_repo")

from contextlib import ExitStack  # noqa: E402

import concourse.bacc as bacc  # noqa: E402
import concourse.mybir as mybir  # noqa: E402
import concourse.tile as tile  # noqa: E402
from concourse.bass_utils import run_bass_kernel_spmd  # noqa: E402

F32 = mybir.dt.float32
F16 = mybir.dt.float16
BF16 = mybir.dt.bfloat16
U16 = mybir.dt.uint16
I16 = mybir.dt.int16
AF = mybir.ActivationFunctionType
ALU = mybir.AluOpType
AX = mybir.AxisListType

N = 1024
K = 20
B = 8
EPS = 1e-5
NEG = -3.0e38
LAYERS = [(3, 64), (64, 64), (64, 128), (128, 256)]  # (C_in, C_out)
NT = N // 128  # 8 m-tiles

ALPHAS = [2.0**21, 2.0**17, 2.0**17, 2.0**17]
DEBUG = bool(int(os.environ.get("KERNEL_DEBUG", "0")))
BIG3 = float(3 * 2**32)                    # binade bias: rounds to 1024
CLAMP0 = float(3 * 2**32 - (2**23 - 1024))  # Relu clamp point
SUBC = float(-(2**23 - 1024))              # post-clamp shift


# ----------------------------------------------------------------- host math
def _fold_bn(w, bn):
    g, b, m, v = [np.asarray(x, np.float32) for x in bn]
    s = (g * (1.0 / np.sqrt(v + EPS))).astype(np.float32)
    A = (s[:, None] * np.asarray(w, np.float32)).astype(np.float32)
    c = (b - m * s).astype(np.float32)
    return A, c


def _build_consts(inp):
    """All per-problem constants, shared by every core. Returns name->array."""
    c = {}
    for li, wkey, bkey in [(0, "w1", "bn1"), (1, "w2", "bn2"),
                           (2, "w3", "bn3"), (3, "w4", "bn4")]:
        Cin, Cout = LAYERS[li]
        A, cc = _fold_bn(inp[wkey], inp[bkey])
        A1, A2 = A[:, :Cin], A[:, Cin:]
        c[f"a1t{li}"] = np.ascontiguousarray(A1.T)            # [Cin, Cout]
        c[f"a2t{li}"] = np.ascontiguousarray(A2.T)            # [Cin, Cout]
        nblk = (Cout + 127) // 128
        c[f"ccol{li}"] = np.ascontiguousarray(
            cc.reshape(nblk, -1).T)                           # [<=128, nblk]
    # conv5
    A5, c5 = _fold_bn(inp["w5"], inp["bn5"])                  # [128, 512]
    ofs = [0, 64, 128, 256, 384, 512]
    for j in range(5):
        c[f"a5t{j}"] = np.ascontiguousarray(A5[:, ofs[j]:ofs[j + 1]].T)
    c["c5col"] = c5.reshape(128, 1).copy()
    # classifier layer 1 (512 <- 256), BN6 + leaky
    A6, c6 = _fold_bn(inp["l1w"], inp["bn6"])                 # [512, 256]
    c["a6at"] = np.ascontiguousarray(A6[:, :128].T)           # [128, 512]
    c["a6bt"] = np.ascontiguousarray((A6[:, 128:] / 1024.0).T)  # [128, 512]
    c["c6"] = np.ascontiguousarray(c6.reshape(4, 128).T)      # [128, 4]
    # classifier layer 2 (256 <- 512), +l2b then BN7 + leaky
    A7, c7 = _fold_bn(inp["l2w"], inp["bn7"])                 # [256, 512]
    s7 = np.asarray(inp["bn7"], np.float32)
    gs7 = (s7[0] * (1.0 / np.sqrt(s7[3] + EPS))).astype(np.float32)
    c7 = (c7 + gs7 * np.asarray(inp["l2b"], np.float32)).astype(np.float32)
    c["a7t"] = np.ascontiguousarray(A7.T.reshape(4, 128, 256).transpose(1, 0, 2))
    c["c7"] = np.ascontiguousarray(c7.reshape(2, 128).T)      # [128, 2]
    # collapse l3/l4/l5 into one affine [40 x 256]
    l3w = np.asarray(inp["l3w"], np.float32); l3b = np.asarray(inp["l3b"], np.float32)
    l4w = np.asarray(inp["l4w"], np.float32); l4b = np.asarray(inp["l4b"], np.float32)
    l5w = np.asarray(inp["l5w"], np.float32); l5b = np.asarray(inp["l5b"], np.float32)
    Wc = (l5w @ l4w @ l3w).astype(np.float32)                 # [40, 256]
    bc = (l5w @ (l4w @ l3b + l4b) + l5b).astype(np.float32)   # [40]
    c["wct"] = np.ascontiguousarray(Wc.T.reshape(2, 128, 40).transpose(1, 0, 2))
    c["cout"] = bc.reshape(40, 1).copy()                      # [40, 1]
    c["ident"] = np.eye(128, dtype=np.float32)
    c["ident16"] = np.eye(128, dtype=np.float16)
    idrep = np.zeros((16, 128), np.float16)                   # replicate %16
    idrep[np.arange(128) % 16, np.arange(128)] = 1.0
    c["idrep16"] = idrep
    c["iotarep"] = np.broadcast_to(
        np.arange(N, dtype=np.float32)[None, :], (128, N)).copy()
    return c


# --------------------------------------------------------------- the program
def _emit(tc, io):
    """Emit the full per-core program. io: name -> DRAM AP."""
    nc = tc.nc
    ctx = ExitStack()

    cp = ctx.enter_context(tc.tile_pool(name="consts", bufs=1))
    hp = ctx.enter_context(tc.tile_pool(name="hbufs", bufs=1))
    sp = ctx.enter_context(tc.tile_pool(name="swork", bufs=2))
    wp = ctx.enter_context(tc.tile_pool(name="work", bufs=2))
    ip = ctx.enter_context(tc.tile_pool(name="idxp", bufs=2))
    vp = ctx.enter_context(tc.tile_pool(name="needles", bufs=3))
    nb = ctx.enter_context(tc.tile_pool(name="nbrp", bufs=3))
    dp = ctx.enter_context(tc.tile_pool(name="dramp", bufs=1, space="DRAM"))
    ps_s = ctx.enter_context(tc.tile_pool(name="ps_s", bufs=2, space="PSUM"))
    ps_u = ctx.enter_context(tc.tile_pool(name="ps_u", bufs=1, space="PSUM"))
    ps_v = ctx.enter_context(tc.tile_pool(name="ps_v", bufs=2, space="PSUM"))
    ps_a = ctx.enter_context(tc.tile_pool(name="ps_a", bufs=1, space="PSUM"))
    ps_r = ctx.enter_context(tc.tile_pool(name="ps_r", bufs=2, space="PSUM"))

    def load_const(name, dt=F32):
        shp = list(io[name].shape)
        t = cp.tile(shp, dt, name=f"c_{name}", tag=f"c_{name}")
        nc.sync.dma_start(t[...], io[name])
        return t

    consts = {}
    for k in io:
        if k in ("hx", "out") or k.startswith("dbg_"):
            continue
        dt = F16 if k in ("ident16", "idrep16") else F32
        consts[k] = load_const(k, dt)
    ones_row = cp.tile([1, 128], F32, name="ones_row", tag="ones_row")
    nc.gpsimd.memset(ones_row[:], 1.0)
    ones_bf = cp.tile([1, 128], BF16, name="ones_bf", tag="ones_bf")
    nc.gpsimd.memset(ones_bf[:], 1.0)
    neghalf = cp.tile([128, 1], F32, name="neghalf", tag="neghalf")
    nc.gpsimd.memset(neghalf[:], -0.5)
    big3col = cp.tile([128, 1], F32, name="big3col", tag="big3col")
    nc.gpsimd.memset(big3col[:], BIG3)
    nclampcol = cp.tile([128, 1], F32, name="nclampcol", tag="nclampcol")
    nc.gpsimd.memset(nclampcol[:], -CLAMP0)
    ident = consts["ident"]
    ident16 = consts["ident16"]
    idrep16 = consts["idrep16"]
    iotarep = consts["iotarep"]

    # h^T buffers, feature-major [C, N]
    hxT = hp.tile([3, N], F32, name="hxT", tag="hxT")
    nc.sync.dma_start(hxT[...], io["hx"])
    h1T = hp.tile([64, N], F32, name="h1T", tag="h1T")
    h2T = hp.tile([64, N], F32, name="h2T", tag="h2T")
    h3T = hp.tile([128, N], F32, name="h3T", tag="h3T")
    h4Ta = hp.tile([128, N], F32, name="h4Ta", tag="h4Ta")
    h4Tb = hp.tile([128, N], F32, name="h4Tb", tag="h4Tb")
    h5T = hp.tile([128, N], F32, name="h5T", tag="h5T")

    h_in = [hxT, h1T, h2T, h3T]
    h_out = [[h1T], [h2T], [h3T], [h4Ta, h4Tb]]

    # ------------------------------------------------------------ edge convs
    for li, (Cin, Cout) in enumerate(LAYERS):
        hT = h_in[li]
        a1t = consts[f"a1t{li}"]
        a2t = consts[f"a2t{li}"]
        ccol = consts[f"ccol{li}"]
        alpha = ALPHAS[li]
        u_dram = dp.tile([N, Cout], F32, name=f"u_dram{li}", tag=f"u_dram{li}")

        # squared norms, feature-major: sq[c, n] = h[c, n]^2
        sq = wp.tile([Cin, N], F32, name=f"sq{li}", tag="sq")
        nc.scalar.activation(sq[...], hT[...], AF.Square)
        # xx row for the column term: xx[n] = -0.5 * sum_c sq[c, n]
        xx_sb = wp.tile([1, N], F32, name=f"xx{li}", tag="xx")
        for nt2 in range(2):
            xx_ps = ps_a.tile([1, 512], F32, name=f"xxps{li}_{nt2}", tag="aux")
            nc.tensor.matmul(xx_ps[...], neghalf[0:Cin, :],
                             sq[:, nt2 * 512:(nt2 + 1) * 512], start=True, stop=True)
            nc.scalar.copy(xx_sb[:, nt2 * 512:(nt2 + 1) * 512], xx_ps[...])
        # per-point row bias: biascol[i] = BIG3 + alpha * (-0.5 xx_i)
        biascol = wp.tile([128, NT], F32, name=f"bcol{li}", tag="bcol")
        for m in range(NT):
            mb = slice(m * 128, (m + 1) * 128)
            bc_ps = ps_a.tile([128, 1], F32, name=f"bcps{li}_{m}", tag="aux")
            nc.tensor.matmul(bc_ps[...], sq[:, mb], neghalf[0:Cin, :],
                             start=True, stop=True)
            nc.scalar.activation(biascol[:, m:m + 1], bc_ps[...], AF.Identity,
                                 scale=float(alpha), bias=big3col[...])

        # hi/lo bf16 split of hT and the xx row: exact to fp32 precision,
        # turns the fp32 LOW_HIGH score matmuls into 1-pass bf16 matmuls.
        if 2 * Cin <= 128:
            BB = 32 if Cin <= 32 else 64
            ha = wp.tile([BB + Cin, N], BF16, name=f"ha{li}", tag="ha")
            ha2 = wp.tile([BB + Cin, N], BF16, name=f"ha2{li}", tag="ha2")
            if Cin < BB:
                nc.gpsimd.memset(ha[...], 0.0)
                nc.gpsimd.memset(ha2[...], 0.0)
            hi_s, lo_s = ha[0:Cin, :], ha[BB:BB + Cin, :]
        else:
            hhi = wp.tile([Cin, N], BF16, name=f"hhi{li}", tag="ha")
            hlo = wp.tile([Cin, N], BF16, name=f"hlo{li}", tag="ha2")
            hi_s, lo_s = hhi[...], hlo[...]
        hi32 = wp.tile([Cin, N], F32, name=f"hi32{li}", tag="hi32")
        nc.scalar.copy(hi_s, hT[...])
        nc.scalar.copy(hi32[...], hi_s)
        nc.vector.tensor_tensor(lo_s, hT[...], hi32[...], ALU.subtract)
        if 2 * Cin <= 128:
            nc.scalar.copy(ha2[0:Cin, :], lo_s)
            nc.scalar.copy(ha2[BB:BB + Cin, :], hi_s)
        xxhi_b = wp.tile([1, N], BF16, name=f"xxhib{li}", tag="xxhib")
        xxlo_b = wp.tile([1, N], BF16, name=f"xxlob{li}", tag="xxlob")
        xxhi32 = wp.tile([1, N], F32, name=f"xxhi32{li}", tag="xxhi32")
        nc.scalar.copy(xxhi_b[...], xx_sb[...])
        nc.scalar.copy(xxhi32[...], xxhi_b[...])
        nc.vector.tensor_tensor(xxlo_b[...], xx_sb[...], xxhi32[...],
                                ALU.subtract)

        # ---------------- phase A per m-tile: embedded scores, top-24, U
        idxs_all = ip.tile([128, NT, 160], I16, name=f"idxsall{li}", tag="idxsall")

        def _half_tail(h0, v24h):
            # index extraction j = wv mod 1024 (fp16) for m-tiles h0..h0+3,
            # then the wrapped idx layout; overlaps the other half's topk.
            vfl = v24h[...].rearrange("p a b -> p (a b)")
            e1 = ip.tile([128, 96], F32, name=f"e1_{li}_{h0}", tag="e1")
            e2 = ip.tile([128, 96], F32, name=f"e2_{li}_{h0}", tag="e2")
            jp = ip.tile([128, 96], F32, name=f"jp_{li}_{h0}", tag="jp")
            mk = ip.tile([128, 96], F32, name=f"mk_{li}_{h0}", tag="mk")
            jf16 = ip.tile([128, 4, 24], F16, name=f"jf16_{li}_{h0}", tag="jf16")
            nc.vector.tensor_scalar(e1[...], vfl, 2.0**-10, 1.5 * 2.0**23,
                                    op0=ALU.mult, op1=ALU.add)
            nc.vector.tensor_scalar(e2[...], e1[...], -1.5 * 2.0**23, -1024.0,
                                    op0=ALU.add, op1=ALU.mult)
            nc.vector.tensor_tensor(jp[...], e2[...], vfl, ALU.add)
            nc.vector.tensor_scalar(mk[...], jp[...], 0.0, None, op0=ALU.is_lt)
            nc.vector.scalar_tensor_tensor(
                jf16[...].rearrange("p a b -> p (a b)"), mk[...], 1024.0,
                jp[...], op0=ALU.mult, op1=ALU.add)
            for g in range(2):          # rep groups of 2 m-tiles
                rep_ps = ps_r.tile([128, 320], F32,
                                   name=f"repps{li}_{h0}_{g}", tag="repall")
                for j in range(2):
                    mm = h0 + g * 2 + j
                    mt_ps = ps_a.tile([20, 128], F16, name=f"mtps{li}_{mm}",
                                      tag="aux")
                    nc.tensor.transpose(mt_ps[...], jf16[:, g * 2 + j, 0:20],
                                        ident16[...])
                    mt_sb = wp.tile([20, 128], F16, name=f"mtsb{li}_{mm}",
                                    tag="mtsb")
                    nc.scalar.copy(mt_sb[...], mt_ps[...])
                    tball = ps_a.tile([16, 8, 20], F16,
                                      name=f"tball{li}_{mm}", tag="aux")
                    for bb in range(8):
                        nc.tensor.transpose(tball[:, bb, :],
                                            mt_sb[:, bb * 16:(bb + 1) * 16],
                                            ident16[0:20, 0:20])
                    wsb = wp.tile([16, 160], F16, name=f"wsb{li}_{mm}",
                                  tag="wsb")
                    nc.scalar.copy(wsb[...].rearrange("p (t b) -> p t b", b=8),
                                   tball[...].rearrange("p b t -> p t b"))
                    nc.tensor.matmul(rep_ps[:, j * 160:(j + 1) * 160],
                                     idrep16[...], wsb[...],
                                     start=True, stop=True)
                nc.scalar.copy(
                    idxs_all[:, h0 + g * 2:h0 + g * 2 + 2, :]
                    .rearrange("p a b -> p (a b)"),
                    rep_ps[...])

        v24h = None
        for m in range(NT):
            mb = slice(m * 128, (m + 1) * 128)
            w0 = sp.tile([128, N], F32, name=f"w0_{li}_{m}", tag="w0")
            rc = sp.tile([128, N], F32, name=f"rc_{li}_{m}", tag="rc")
            wv = sp.tile([128, N], F32, name=f"wv_{li}_{m}", tag="wv")
            for nt2 in range(2):
                ns = slice(nt2 * 512, (nt2 + 1) * 512)
                s_ps = ps_s.tile([128, 512], F32, name=f"sps{li}_{m}_{nt2}", tag="sps")
                if 2 * Cin <= 128:
                    nc.tensor.matmul(s_ps[...], ha[:, mb], ha[:, ns],
                                     start=True, stop=False)
                    nc.tensor.matmul(s_ps[...], ha[:, mb], ha2[:, ns],
                                     start=False, stop=False,
                                     skip_group_check=True)
                else:
                    nc.tensor.matmul(s_ps[...], hhi[:, mb], hhi[:, ns],
                                     start=True, stop=False)
                    nc.tensor.matmul(s_ps[...], hhi[:, mb], hlo[:, ns],
                                     start=False, stop=False,
                                     skip_group_check=True)
                    nc.tensor.matmul(s_ps[...], hlo[:, mb], hhi[:, ns],
                                     start=False, stop=False,
                                     skip_group_check=True)
                nc.tensor.matmul(s_ps[...], ones_bf[...], xxhi_b[:, ns],
                                 start=False, stop=False, skip_group_check=True)
                nc.tensor.matmul(s_ps[...], ones_bf[...], xxlo_b[:, ns],
                                 start=False, stop=True, skip_group_check=True)
                # w0 = fp32(alpha*s + biascol) -- rounds to 1024 in 2^33 binade
                nc.scalar.activation(w0[:, ns], s_ps[...], AF.Identity,
                                     scale=float(alpha), bias=biascol[:, m:m + 1])
            # clamp far candidates, shift near zero
            nc.scalar.activation(rc[...], w0[...], AF.Relu, bias=nclampcol[...])
            # embed index j in the low bits
            nc.vector.scalar_tensor_tensor(wv[...], rc[...], SUBC, iotarep[...],
                                           op0=ALU.add, op1=ALU.add)
            if DEBUG and li == 0 and m == 0:
                nc.sync.dma_start(io["dbg_w0"], w0[...])
                nc.sync.dma_start(io["dbg_wv"], wv[...])
            # top-24 via 3 rounds of max8 + match_replace (no index scans)
            if m % 4 == 0:
                v24h = ip.tile([128, 4, 24], F32, name=f"v24_{li}_{m}",
                               tag="v24h")
            for r in range(3):
                v8 = v24h[:, m % 4, r * 8:(r + 1) * 8]
                nc.vector.max(v8, wv[...])
                if r < 2:
                    nc.vector.match_replace(wv[...], v8, wv[...], NEG)

            # U tile: [128 pts, Cout] point-major, stored to DRAM for the gather
            u_ps = ps_u.tile([128, Cout], F32, name=f"ups{li}_{m}", tag="ups")
            nc.tensor.matmul(u_ps[...], hT[:, mb], a1t[...], start=True, stop=True)
            u_sb = wp.tile([128, Cout], F32, name=f"usb{li}_{m}", tag="usb")
            nc.scalar.copy(u_sb[...], u_ps[...])
            nc.sync.dma_start(u_dram[mb, :], u_sb[...])

            if m % 4 == 3:
                _half_tail(m - 3, v24h)


        # ---------------- phase B per m-tile: gather, k-max, combine on PE/ACT
        for m in range(NT):
            mb = slice(m * 128, (m + 1) * 128)
            nbr = nb.tile([128, K, Cout], F32, name=f"nbr{li}_{m}", tag="nbr")
            nh = 128 * K // 2
            nc.gpsimd.dma_gather(nbr[:, 0:K // 2, :], u_dram[...],
                                 idxs_all[:, m, 0:80], num_idxs=nh,
                                 num_idxs_reg=nh, elem_size=Cout,
                                 single_packet=False, queue_num=(2 * m) % 4)
            nc.gpsimd.dma_gather(nbr[:, K // 2:K, :], u_dram[...],
                                 idxs_all[:, m, 80:160], num_idxs=nh,
                                 num_idxs_reg=nh, elem_size=Cout,
                                 single_packet=False, queue_num=(2 * m + 1) % 4)
            mx = wp.tile([128, Cout], F32, name=f"mx{li}_{m}", tag="mx")
            mx2 = wp.tile([128, Cout], F32, name=f"mx2{li}_{m}", tag="mx2")
            nc.vector.tensor_reduce(
                mx[...], nbr[:, 0:K // 2, :].rearrange("p t c -> p c t"),
                axis=AX.X, op=ALU.max)
            nc.vector.tensor_reduce(
                mx2[...], nbr[:, K // 2:K, :].rearrange("p t c -> p c t"),
                axis=AX.X, op=ALU.max)
            nc.vector.tensor_tensor(mx[...], mx[...], mx2[...], ALU.max)
            if DEBUG and li == 0 and m == 0:
                nc.sync.dma_start(io["dbg_nbr"], nbr[...])
                nc.sync.dma_start(io["dbg_mx"], mx[...])

            # zT = transpose(mx) + A2 h  (PSUM accumulate), h = Lrelu(zT + c)
            for ci, hdst in enumerate(h_out[li]):
                cs = slice(ci * 128, min((ci + 1) * 128, Cout))
                w = cs.stop - cs.start
                z_ps = ps_v.tile([w, 128], F32, name=f"zps{li}_{m}_{ci}",
                                 tag="vps")
                nc.tensor.matmul(z_ps[...], mx[:, cs], ident[...],
                                 is_transpose=True, start=True, stop=False)
                nc.tensor.matmul(z_ps[...], a2t[:, cs], hT[:, mb],
                                 start=False, stop=True, skip_group_check=True)
                zsb = wp.tile([w, 128], F32, name=f"zsb{li}_{m}_{ci}", tag="zsb")
                nc.scalar.activation(zsb[...], z_ps[...], AF.Identity,
                                     bias=ccol[0:w, ci:ci + 1])
                nc.vector.scalar_tensor_tensor(hdst[0:w, mb], zsb[...], 0.2,
                                               zsb[...], op0=ALU.mult,
                                               op1=ALU.max)

    if DEBUG:
        nc.sync.dma_start(io["dbg_h1"], h1T[...])
        nc.sync.dma_start(io["dbg_h4a"], h4Ta[...])

    # ------------------------------------------------------------ conv5
    a5 = [consts[f"a5t{j}"] for j in range(5)]
    srcs = [h1T, h2T, h3T, h4Ta, h4Tb]
    for nt2 in range(2):
        ns = slice(nt2 * 512, (nt2 + 1) * 512)
        h5_ps = ps_s.tile([128, 512], F32, name=f"h5ps{nt2}", tag="sps")
        for j in range(5):
            nc.tensor.matmul(h5_ps[...], a5[j][...], srcs[j][:, ns],
                             start=(j == 0), stop=(j == 4))
        zt = sp.tile([128, 512], F32, name=f"h5z{nt2}", tag="w0")
        nc.scalar.activation(zt[...], h5_ps[...], AF.Identity,
                             bias=consts["c5col"][...])
        nc.vector.scalar_tensor_tensor(h5T[:, ns], zt[...], 0.2, zt[...],
                                       op0=ALU.mult, op1=ALU.max)

    if DEBUG:
        nc.sync.dma_start(io["dbg_h5"], h5T[...])

    # ------------------------------------------------------------ pooling
    gmax = wp.tile([128, 1], F32, name="gmax", tag="gpool")
    nc.vector.tensor_reduce(gmax[...], h5T[...], axis=AX.X, op=ALU.max)
    gsum = wp.tile([128, 1], F32, name="gsum", tag="gpool")
    nc.vector.tensor_reduce(gsum[...], h5T[...], axis=AX.X, op=ALU.add)

    # ------------------------------------------------------------ classifier
    a6at, a6bt, c6 = consts["a6at"], consts["a6bt"], consts["c6"]
    y1 = wp.tile([128, 4], F32, name="y1", tag="y1")
    for mt in range(4):
        ms = slice(mt * 128, (mt + 1) * 128)
        y_ps = ps_v.tile([128, 1], F32, name=f"y1ps{mt}", tag="vps")
        nc.tensor.matmul(y_ps[...], a6at[:, ms], gmax[...], start=True, stop=False)
        nc.tensor.matmul(y_ps[...], a6bt[:, ms], gsum[...], start=False, stop=True)
        nc.scalar.activation(y1[:, mt:mt + 1], y_ps[...], AF.Identity,
                             bias=c6[:, mt:mt + 1])
    y1l = wp.tile([128, 4], F32, name="y1l", tag="y1")
    nc.vector.scalar_tensor_tensor(y1l[...], y1[...], 0.2, y1[...],
                                   op0=ALU.mult, op1=ALU.max)

    a7t, c7 = consts["a7t"], consts["c7"]
    y2 = wp.tile([128, 2], F32, name="y2", tag="y2")
    for m2 in range(2):
        ms = slice(m2 * 128, (m2 + 1) * 128)
        y_ps = ps_v.tile([128, 1], F32, name=f"y2ps{m2}", tag="vps")
        for kc in range(4):
            nc.tensor.matmul(y_ps[...], a7t[:, kc, ms], y1l[:, kc:kc + 1],
                             start=(kc == 0), stop=(kc == 3))
        nc.scalar.activation(y2[:, m2:m2 + 1], y_ps[...], AF.Identity,
                             bias=c7[:, m2:m2 + 1])
    y2l = wp.tile([128, 2], F32, name="y2l", tag="y2")
    nc.vector.scalar_tensor_tensor(y2l[...], y2[...], 0.2, y2[...],
                                   op0=ALU.mult, op1=ALU.max)

    wct, cout = consts["wct"], consts["cout"]
    y5_ps = ps_v.tile([40, 1], F32, name="y5ps", tag="vps")
    for kc in range(2):
        nc.tensor.matmul(y5_ps[...], wct[:, kc, :], y2l[:, kc:kc + 1],
                         start=(kc == 0), stop=(kc == 1))
    y5 = wp.tile([40, 1], F32, name="y5", tag="y5")
    nc.scalar.activation(y5[...], y5_ps[...], AF.Identity, bias=cout[...])
    nc.sync.dma_start(io["out"], y5[...])

    ctx.close()


def _install_profile_hook():
    """The agent image's antenv lacks axon_hooks; recreate it so trace=True
    can drive NTFF profiling through libaxon_pjrt.so (test-only path)."""
    import types
    try:
        from antenv.axon_hooks import get_axon_ntff_profile_hook  # noqa: F401
        return
    except ImportError:
        pass
    mod = types.ModuleType("antenv.axon_hooks")
    _h = [None]
    mod.set_axon_ntff_profile_hook = lambda h: _h.__setitem__(0, h)
    mod.get_axon_ntff_profile_hook = lambda: _h[0]
    import antenv
    antenv.axon_hooks = mod
    sys.modules["antenv.axon_hooks"] = mod
    if "/root/.axon_site" not in sys.path:
        sys.path.insert(0, "/root/.axon_site")
    from trn_agent_boot.trn_boot import _ntff_profile_via_ctypes
    mod.set_axon_ntff_profile_hook(
        _ntff_profile_via_ctypes("/opt/axon/libaxon_pjrt.so"))
    import concourse.bass_utils as _bu
    _bu.upload_artifacts = lambda tmpdir: tmpdir


# --------------------------------------------------------------- build + run
_CACHE = {}


def _build_program(const_shapes):
    nc = bacc.Bacc("TRN2", target_bir_lowering=False, debug=False,
                   enable_asserts=False, num_devices=B, num_swdge_queues=4)
    io = {}
    io["hx"] = nc.dram_tensor("hx", [3, N], F32, kind="ExternalInput").ap()
    for name, shp in const_shapes.items():
        dt = F16 if name in ("ident16", "idrep16") else F32
        io[name] = nc.dram_tensor(name, list(shp), dt, kind="ExternalInput").ap()
    io["out"] = nc.dram_tensor("out", [40], F32, kind="ExternalOutput").ap()
    if DEBUG:
        for nm, shp, dt in [("dbg_w0", [128, N], F32), ("dbg_wv", [128, N], F32),
                            ("dbg_v24", [128, NT, 24], F32),
                            ("dbg_jf", [128, NT, 24], F16),
                            ("dbg_idxs", [128, NT, 160], I16),
                            ("dbg_udram", [N, 64], F32),
                            ("dbg_bcol", [128, NT], F32),
                            ("dbg_nbr", [128, K, 64], F32),
                            ("dbg_mx", [128, 64], F32),
                            ("dbg_h1", [64, N], F32),
                            ("dbg_h4a", [128, N], F32),
                            ("dbg_h5", [128, N], F32)]:
            io[nm] = nc.dram_tensor(nm, shp, dt, kind="ExternalOutput").ap()
    with tile.TileContext(nc) as tc:
        _emit(tc, io)
    nc.compile()
    return nc


def kernel(**inputs):
    consts = _build_consts(inputs)
    key = "prog"
    if key not in _CACHE:
        _CACHE[key] = _build_program({k: v.shape for k, v in consts.items()})
    nc = _CACHE[key]

    x = np.asarray(inputs["x"], np.float32)
    in_maps = []
    for bi in range(B):
        m = {"hx": np.ascontiguousarray(x[bi])}
        m.update(consts)
        in_maps.append(m)

    trace = bool(int(os.environ.get("KERNEL_TRACE", "0")))
    if trace:
        _install_profile_hook()
    res = run_bass_kernel_spmd(nc, in_maps, core_ids=list(range(B)), trace=trace)
    kernel.last_result = res
    out = np.stack([r["out"] for r in res.results], axis=0).astype(np.float32)
    return out


if __name__ == "__main__":
    import reference as R
    inp = {k: np.asarray(v) for k, v in R.setup_inputs().items()}
    got = kernel(**inp)
    exp = np.asarray(R.reference(**R.setup_inputs()))
    err = np.abs(got - exp).max() / np.abs(exp).max()
    print("rel err:", err)

